# revision 1
# baseline (speedup 1.0000x reference)
"""Causal self-attention (B=4, T=2048, C=1024, H=16, D=64) on 8 TRN2 NeuronCores.

Sharding: data-parallel over batch (4) x tensor-parallel over heads (2 groups
of 8 heads).  Core c handles batch c//2 and heads (c%2)*8 .. (c%2)*8+8.
Each core computes its QKV projection shard, causal flash-style attention for
its 8 heads, and a partial output projection (row-parallel).  The host sums
the two partials per batch and adds b_proj.

Device layouts (per core):
  xT   [1024, 2048]  x[b].T (channels on partitions), bf16
  wqk  [1025, 1024]  [Wq | Wk] columns for this head group; row 1024 = bias
  wv   [1025,  512]  Wv columns; row 1024 = bias
  wp   [ 512, 1024]  w_proj rows for this head group
  masks[ 128, 2048]  4 diagonal causal masks (kv-local x query-local)
  outT [1024, 2048]  partial (attn @ wp).T, before b_proj, fp32

All matmuls run in bf16 with fp32 PSUM accumulation.  Attention scores are
computed transposed (kv on partitions, queries free) so the softmax
probabilities feed the AV matmul directly as the moving operand; the
denominator comes for free from a ones column appended to V.  All pools live
in one scope so the Tile scheduler overlaps QKV, attention and projection.
"""

import os
import sys

for _p in (
    "/root/.axon_site",
    "/root/.axon_site/_ro/trn_rl_repo",
    "/root/.axon_site/_ro/pypackages",
    "/opt/trn_rl_repo",
):
    if os.path.isdir(_p) and _p not in sys.path:
        sys.path.append(_p)

from contextlib import ExitStack

import ml_dtypes
import numpy as np

import concourse.tile as tile
from concourse import bacc, mybir
from concourse.bass import ds, ts
from concourse.bass_utils import run_bass_kernel_spmd

F32 = mybir.dt.float32
F32R = mybir.dt.float32r
BF16 = mybir.dt.bfloat16
EXP = mybir.ActivationFunctionType.Exp
MULT = mybir.AluOpType.mult

B, T, C, H, D = 4, 2048, 1024, 16, 64
HPC = 8            # heads per core
CS = HPC * D       # 512 sharded channels
NC_ = C // 128     # 8 channel tiles
TB = T // 128      # 16 token blocks
TCH = T // 512     # 4 query chunks
SCALE = 1.0 / np.sqrt(D)

_CACHE = {}


def _build_program():
    nc = bacc.Bacc("TRN2", target_bir_lowering=False, debug=False)

    xT_d = nc.dram_tensor("xT", [C, T], BF16, kind="ExternalInput")
    wqk_d = nc.dram_tensor("wqk", [C + 1, 2 * CS], BF16, kind="ExternalInput")
    wv_d = nc.dram_tensor("wv", [C + 1, CS], BF16, kind="ExternalInput")
    wp_d = nc.dram_tensor("wp", [CS, C], BF16, kind="ExternalInput")
    mk_d = nc.dram_tensor("masks", [128, 2048], BF16, kind="ExternalInput")
    on_d = nc.dram_tensor("ones", [128, 512], BF16, kind="ExternalInput")
    outT_d = nc.dram_tensor("outT", [C, T], F32, kind="ExternalOutput")

    with tile.TileContext(nc) as tc, ExitStack() as ctx, \
            nc.allow_low_precision(reason="bf16 matmuls, f32r epilogue"):
        pool_c = ctx.enter_context(tc.tile_pool(name="const", bufs=1))
        pool_qk = ctx.enter_context(tc.tile_pool(name="qkT", bufs=1))
        pool_ve = ctx.enter_context(tc.tile_pool(name="vext", bufs=1))
        pool_x = ctx.enter_context(tc.tile_pool(name="xT", bufs=1))
        pool_w = ctx.enter_context(tc.tile_pool(name="wqk", bufs=2))
        pool_wv = ctx.enter_context(tc.tile_pool(name="wv", bufs=1))
        pool_p = ctx.enter_context(tc.tile_pool(name="P", bufs=4))
        pool_r = ctx.enter_context(tc.tile_pool(name="rec", bufs=3))
        pool_tmp = ctx.enter_context(tc.tile_pool(name="psb", bufs=14))
        pool_y = ctx.enter_context(tc.tile_pool(name="yT", bufs=1))
        pool_wp = ctx.enter_context(tc.tile_pool(name="wp", bufs=1))
        pool_o = ctx.enter_context(tc.tile_pool(name="out", bufs=3))
        ps_mm = ctx.enter_context(tc.tile_pool(name="psmm", bufs=3, space="PSUM"))
        ps_y = ctx.enter_context(tc.tile_pool(name="psy", bufs=2, space="PSUM"))

        ones = pool_c.tile([128, 512], BF16)
        nc.sync.dma_start(ones[:], on_d.ap()[:])

        # first m-tile's weights before the big xT load so matmuls start early
        w0_sb = pool_w.tile([128, NC_, 128], BF16, tag="wqk", name="w0_sb")
        nc.sync.dma_start(
            w0_sb[:],
            wqk_d.ap()[0:C, ts(0, 128)].rearrange("(c p) f -> p c f", p=128),
        )
        bq0_sb = pool_w.tile([1, 128], BF16, tag="bqk", name="bq0_sb")
        nc.sync.dma_start(bq0_sb[:], wqk_d.ap()[C : C + 1, ts(0, 128)])

        # xT split per channel tile: QKV matmuls depend only on their slice
        xT = pool_x.tile([128, NC_, T], BF16)
        xT_ap = xT_d.ap().rearrange("(c p) t -> p c t", p=128)
        for ci in range(NC_):
            nc.sync.dma_start(xT[:, ci], xT_ap[:, ci])

        wv_sb = pool_wv.tile([128, NC_, CS], BF16)
        nc.sync.dma_start(
            wv_sb[:], wv_d.ap()[0:C].rearrange("(c p) f -> p c f", p=128)
        )
        bv_sb = pool_wv.tile([1, CS], BF16)
        nc.sync.dma_start(bv_sb[:], wv_d.ap()[C : C + 1])

        # qkT[p, m, t]: feature-major q|k activations, feature f = m*128+p
        qkT = pool_qk.tile([128, 2 * CS // 128, T], BF16)
        # v_ext[p, tb, h*65+d]; col h*65+64 holds ones (the denominator trick)
        vext = pool_ve.tile([128, TB, HPC * 65], BF16)
        nc.sync.dma_start(
            vext[:].rearrange("p tb (h s) -> p tb h s", s=65)[:, :, :, 64],
            on_d.ap()[:, 0 : TB * HPC].rearrange("p (tb h) -> p tb h", h=HPC),
        )
        masks = pool_c.tile([128, 4, 512], BF16)
        nc.sync.dma_start(masks[:], mk_d.ap().rearrange("p (r i) -> p r i", r=4))
        yT = pool_y.tile([128, CS // 128, T], BF16)

        # ---------------- QKV projection ----------------
        def qk_mtile(m, w_sb=None, bq_sb=None):
            if w_sb is None:
                w_sb = pool_w.tile([128, NC_, 128], BF16, tag="wqk")
                nc.sync.dma_start(
                    w_sb[:],
                    wqk_d.ap()[0:C, ts(m, 128)].rearrange("(c p) f -> p c f",
                                                          p=128),
                )
                bq_sb = pool_w.tile([1, 128], BF16, tag="bqk")
                nc.sync.dma_start(bq_sb[:], wqk_d.ap()[C : C + 1, ts(m, 128)])
            for tch in range(TCH):
                ps = ps_mm.tile([128, 1024], F32, tag="mm", name="qkps")
                for ci in range(NC_):
                    nc.tensor.matmul(
                        ps[:, 0:512], w_sb[:, ci], xT[:, ci, ts(tch, 512)],
                        start=(ci == 0), stop=False,
                    )
                nc.tensor.matmul(
                    ps[:, 0:512], bq_sb[:], ones[0:1, :], start=False, stop=True
                )
                nc.any.tensor_copy(out=qkT[:, m, ts(tch, 512)], in_=ps[:, 0:512])

        def v_phase():
            for tb in range(TB):
                ps = ps_mm.tile([128, 1024], F32, tag="mm", name="vps")
                for ci in range(NC_):
                    nc.tensor.matmul(
                        ps[:, 0:512], xT[:, ci, ts(tb, 128)], wv_sb[:, ci],
                        start=(ci == 0), stop=False,
                    )
                nc.tensor.matmul(
                    ps[:, 0:512], ones[0:1, 0:128], bv_sb[:],
                    start=False, stop=True,
                )
                nc.any.tensor_copy(
                    out=vext[:, tb].rearrange("p (h s) -> p h s", s=65)[:, :, 0:64],
                    in_=ps[:, 0:512].rearrange("p (h d) -> p h d", d=64),
                )

        # ---------------- attention for one head pair ----------------
        def attn_pair(p):
            den2 = pool_r.tile([32, 128], F32R, tag="den2")
            psbs = {}
            for I in range(TCH):
                nj = 4 * I + 4
                psy = [
                    ps_y.tile([65, 512], F32, tag="psy", name=f"psy{hb_}")
                    for hb_ in range(2)
                ]
                for jp in range(nj // 2):
                    pss = [
                        ps_mm.tile([128, 1024], F32, tag="mm", name=f"pss{hb_}")
                        for hb_ in range(2)
                    ]
                    for hb in range(2):
                        base = hb * 64
                        for jj in range(2):
                            j = 2 * jp + jj
                            nc.tensor.matmul(
                                pss[hb][:, ts(jj, 512)],
                                qkT[base : base + 64, 4 + p, ts(j, 128)],
                                qkT[base : base + 64, p, ds(I * 512, 512)],
                            )
                    P = [
                        pool_p.tile([128, 1024], BF16, tag="P", name=f"P{hb_}")
                        for hb_ in range(2)
                    ]
                    for hb in range(2):
                        nc.scalar.activation(
                            P[hb][:], pss[hb][:], EXP, scale=float(SCALE)
                        )
                    for jj in range(2):
                        r = 2 * jp + jj - 4 * I
                        if r >= 0:
                            for hb in range(2):
                                nc.vector.tensor_tensor(
                                    P[hb][:, ts(jj, 512)],
                                    P[hb][:, ts(jj, 512)],
                                    masks[:, r],
                                    MULT,
                                )
                    for jj in range(2):
                        j = 2 * jp + jj
                        for hb in range(2):
                            h = 2 * p + hb
                            nc.tensor.matmul(
                                psy[hb][:],
                                vext[:, j, ds(h * 65, 65)],
                                P[hb][:, ts(jj, 512)],
                                start=(j == 0),
                                stop=(j == nj - 1),
                            )
                # free the AV psum banks fast: copy to SBUF (kept until pair
                # end) and scatter the denominator row into den2 (4 rows x
                # 128) for one batched reciprocal.
                for hb in range(2):
                    psb = pool_tmp.tile(
                        [65, 512], F32R, tag="psysb", name=f"psb{hb}"
                    )
                    nc.vector.tensor_copy(out=psb[:], in_=psy[hb][:])
                    g = (I * 2 + hb) * 4
                    nc.sync.dma_start(den2[g : g + 4, :], psb[64:65, :])
                    psbs[(I, hb)] = psb
            # pair-end epilogue: one reciprocal for all 8 denominator rows,
            # gpsimd-broadcast each recip row, normalize on lanes 0..63, DMA
            # into yT.  Overlaps the next pair's attention.
            rec2 = pool_r.tile([32, 128], F32R, tag="rec2")
            nc.vector.reciprocal(rec2[:], den2[:])
            for I in range(TCH):
                for hb in range(2):
                    g = (I * 2 + hb) * 4
                    rec0 = pool_r.tile([1, 512], F32R, tag="rec0")
                    nc.sync.dma_start(rec0[:], rec2[g : g + 4, :])
                    bc = pool_r.tile([64, 512], F32R, tag="bc")
                    nc.gpsimd.partition_broadcast(bc[:], rec0[:])
                    yn = pool_tmp.tile([64, 512], BF16, tag="yn")
                    nc.vector.tensor_tensor(
                        yn[:], psbs[(I, hb)][0:64, :], bc[:], MULT
                    )
                    nc.sync.dma_start(
                        yT[hb * 64 : hb * 64 + 64, p, ds(I * 512, 512)], yn[:]
                    )

        # ---------------- output projection (one m-tile) ----------------
        wp_sb = pool_wp.tile([128, CS // 128, C], BF16)
        outT_ap = outT_d.ap().rearrange("(co p) t -> p co t", p=128)

        def proj_co(co):
            for tch in range(TCH):
                ps = ps_mm.tile([128, 1024], F32, tag="mm", name="projps")
                for cit in range(CS // 128):
                    nc.tensor.matmul(
                        ps[:, 0:512],
                        wp_sb[:, cit, ts(co, 128)],
                        yT[:, cit, ts(tch, 512)],
                        start=(cit == 0),
                        stop=(cit == CS // 128 - 1),
                    )
                ot = pool_o.tile([128, 512], F32, tag="out")
                nc.any.tensor_copy(out=ot[:], in_=ps[:, 0:512])
                nc.sync.dma_start(outT_ap[:, co, ts(tch, 512)], ot[:])

        # emission order = scheduling priority: QKV for pair 0 first, then
        # interleave remaining QKV m-tiles with attention pairs so ACT/DVE
        # softmax work overlaps the PE-heavy projection phases.
        qk_mtile(0, w0_sb, bq0_sb)
        qk_mtile(4)
        v_phase()
        attn_pair(0)
        qk_mtile(1)
        qk_mtile(5)
        attn_pair(1)
        qk_mtile(2)
        qk_mtile(6)
        attn_pair(2)
        qk_mtile(3)
        qk_mtile(7)
        nc.sync.dma_start(
            wp_sb[:], wp_d.ap().rearrange("(c p) f -> p c f", p=128)
        )
        attn_pair(3)
        for co in range(C // 128):
            proj_co(co)

    nc.compile()
    return nc


def _masks_host() -> np.ndarray:
    # masks[p, r*512 + i] = 1.0 if i >= r*128 + p else 0.0
    p = np.arange(128)[:, None]
    i = np.arange(512)[None, :]
    out = np.empty((128, 4, 512), dtype=np.float32)
    for r in range(4):
        out[:, r, :] = (i >= r * 128 + p).astype(np.float32)
    return out.reshape(128, 2048)


def kernel(x, w_qkv, b_qkv, w_proj, b_proj):
    x = np.asarray(x, dtype=np.float32)
    w_qkv = np.asarray(w_qkv, dtype=np.float32)
    b_qkv = np.asarray(b_qkv, dtype=np.float32)
    w_proj = np.asarray(w_proj, dtype=np.float32)
    b_proj = np.asarray(b_proj, dtype=np.float32)

    if "nc" not in _CACHE:
        _CACHE["nc"] = _build_program()
    nc = _CACHE["nc"]

    bf = ml_dtypes.bfloat16
    masks = _masks_host().astype(bf)
    ones = np.ones((128, 512), dtype=bf)

    in_maps = []
    for c in range(8):
        b, hg = c // 2, c % 2
        sl = slice(hg * CS, (hg + 1) * CS)
        wq = np.concatenate(
            [w_qkv[:, sl], w_qkv[:, C + hg * CS : C + (hg + 1) * CS]], axis=1
        )
        bq = np.concatenate([b_qkv[sl], b_qkv[C + hg * CS : C + (hg + 1) * CS]])
        wv = w_qkv[:, 2 * C + hg * CS : 2 * C + (hg + 1) * CS]
        bv = b_qkv[2 * C + hg * CS : 2 * C + (hg + 1) * CS]
        in_maps.append({
            "xT": np.ascontiguousarray(x[b].T).astype(bf),
            "wqk": np.concatenate([wq, bq[None, :]], axis=0).astype(bf),
            "wv": np.concatenate([wv, bv[None, :]], axis=0).astype(bf),
            "wp": np.ascontiguousarray(w_proj[hg * CS : (hg + 1) * CS]).astype(bf),
            "masks": masks,
            "ones": ones,
        })

    _CACHE["in_maps"] = in_maps
    res = run_bass_kernel_spmd(nc, in_maps, core_ids=list(range(8)))

    out = np.empty((B, T, C), dtype=np.float32)
    for b in range(B):
        out[b] = res.results[2 * b]["outT"].T
        out[b] += res.results[2 * b + 1]["outT"].T
        out[b] += b_proj
    return out



# revision 8
# speedup vs baseline: 1.0124x; 1.0124x over previous
"""Causal self-attention (B=4, T=2048, C=1024, H=16, D=64) on 8 TRN2 NeuronCores.

Sharding: data-parallel over batch (4) x tensor-parallel over heads (2 groups
of 8 heads).  Core c handles batch c//2 and heads (c%2)*8 .. (c%2)*8+8.
Each core computes its QKV projection shard, causal attention for its 8
heads, and a partial output projection (row-parallel); the host sums the two
partials per batch and adds b_proj (+ the folded V-bias term b_v @ w_proj).

Schedule: attention runs chunk-outer (query chunk I of 512) / pair-inner
(4 head pairs), emitted as software-pipelined groups (scores -> exp -> mask ->
AV).  Between groups, independent "fill" matmuls (QKV m-tiles, V blocks,
output-projection tiles) are dripped in by a cycle-debt counter so the PE
in-order queue never starves while ACT evaluates exp -- keeping the HAM clock
gate warm (2.4 GHz).  Diagonal blocks are computed at 128-column granularity
(widths 512/384/256/128) with a single shared [128,128] triangular mask.

Device layouts (per core, all contiguous per partition for fat DMA packets):
  xT   [128, 8, 2048]  x[b].T tiled: [p, ci, t], channel = ci*128+p, bf16
  wqk  [128, 8, 8, 128] [p, m, ci, f]: q|k weight columns, feature = m*128+f
  bqk  [128, 8]        q|k bias, feature m*128+p (per-partition for DVE add)
  wv   [128, 8, 512]   [p, ci, f] V weight columns for this head group
  wp   [128, 4, 1024]  [p, cit, f] w_proj rows (head group), row = cit*128+p
  tri  [128, 128]      tri[p, i] = 1 if i >= p (causal mask for diag blocks)
  outT [1024, 2048]    partial (attn @ wp).T before bias, fp32

All matmuls bf16 with fp32 PSUM accumulation.  Scores are computed transposed
(kv on partitions, queries free) so exp(P) feeds the AV matmul directly as
the moving operand; the denominator comes from a ones column in vext.
"""

import os
import sys

for _p in (
    "/root/.axon_site",
    "/root/.axon_site/_ro/trn_rl_repo",
    "/root/.axon_site/_ro/pypackages",
    "/opt/trn_rl_repo",
):
    if os.path.isdir(_p) and _p not in sys.path:
        sys.path.append(_p)

from collections import deque
from contextlib import ExitStack

import ml_dtypes
import numpy as np

import concourse.tile as tile
from concourse import bacc, mybir
from concourse.bass import ds, ts
from concourse.bass_utils import run_bass_kernel_spmd

F32 = mybir.dt.float32
F32R = mybir.dt.float32r
BF16 = mybir.dt.bfloat16
EXP = mybir.ActivationFunctionType.Exp
MULT = mybir.AluOpType.mult

B, T, C, H, D = 4, 2048, 1024, 16, 64
HPC = 8            # heads per core
CS = HPC * D       # 512 sharded channels
NC_ = C // 128     # 8 channel tiles
TB = T // 128      # 16 key blocks
TCH = T // 512     # 4 query chunks
SCALE = 1.0 / np.sqrt(D)

_CACHE = {}


def _build_program():
    nc = bacc.Bacc("TRN2", target_bir_lowering=False, debug=False)

    xT_d = nc.dram_tensor("xT", [128, NC_, T], BF16, kind="ExternalInput")
    wqk_d = nc.dram_tensor("wqk", [128, 8, NC_, 128], BF16, kind="ExternalInput")
    bqk_d = nc.dram_tensor("bqk", [128, 8], F32, kind="ExternalInput")
    wv_d = nc.dram_tensor("wv", [128, NC_, CS], BF16, kind="ExternalInput")
    wp_d = nc.dram_tensor("wp", [128, CS // 128, C], BF16, kind="ExternalInput")
    tri_d = nc.dram_tensor("tri", [128, 128], BF16, kind="ExternalInput")
    outT_d = nc.dram_tensor("outT", [C, T], F32, kind="ExternalOutput")

    with tile.TileContext(nc) as tc, ExitStack() as ctx, \
            nc.allow_low_precision(reason="bf16 matmuls, f32r epilogue"):
        pool_c = ctx.enter_context(tc.tile_pool(name="const", bufs=1))
        pool_x = ctx.enter_context(tc.tile_pool(name="xT", bufs=1))
        pool_w = ctx.enter_context(tc.tile_pool(name="wqk", bufs=1))
        pool_wv = ctx.enter_context(tc.tile_pool(name="wv", bufs=1))
        pool_wp = ctx.enter_context(tc.tile_pool(name="wp", bufs=1))
        pool_qk = ctx.enter_context(tc.tile_pool(name="qkT", bufs=1))
        pool_ve = ctx.enter_context(tc.tile_pool(name="vext", bufs=1))
        pool_y = ctx.enter_context(tc.tile_pool(name="yT", bufs=1))
        pool_p = ctx.enter_context(tc.tile_pool(name="P", bufs=4))
        pool_sb = ctx.enter_context(tc.tile_pool(name="psb", bufs=4))
        pool_rc = ctx.enter_context(tc.tile_pool(name="rec", bufs=2))
        pool_bc = ctx.enter_context(tc.tile_pool(name="bc", bufs=2))
        pool_yn = ctx.enter_context(tc.tile_pool(name="yn", bufs=2))
        pool_o = ctx.enter_context(tc.tile_pool(name="out", bufs=3))
        ps_big = ctx.enter_context(tc.tile_pool(name="psbig", bufs=2, space="PSUM"))
        ps_sm = ctx.enter_context(tc.tile_pool(name="pssm", bufs=2, space="PSUM"))
        ps_y = ctx.enter_context(tc.tile_pool(name="psy", bufs=2, space="PSUM"))

        tri = pool_c.tile([128, 128], BF16)
        nc.sync.dma_start(tri[:], tri_d.ap()[:])
        bqk = pool_c.tile([128, 8], F32)
        nc.sync.dma_start(bqk[:], bqk_d.ap()[:])

        wqk = pool_w.tile([128, 8, NC_, 128], BF16)
        for m in (0, 4):
            nc.sync.dma_start(wqk[:, m], wqk_d.ap()[:, m])
        wv = pool_wv.tile([128, NC_, CS], BF16)
        nc.sync.dma_start(wv[:], wv_d.ap()[:])
        xT = pool_x.tile([128, NC_, T], BF16)
        for tch in range(TCH):
            for ci in range(NC_):
                nc.sync.dma_start(
                    xT[:, ci, ts(tch, 512)], xT_d.ap()[:, ci, ts(tch, 512)]
                )
        for m in (1, 5, 2, 6, 3, 7):
            nc.sync.dma_start(wqk[:, m], wqk_d.ap()[:, m])
        wp = pool_wp.tile([128, CS // 128, C], BF16)
        nc.sync.dma_start(wp[:], wp_d.ap())

        qkT = pool_qk.tile([128, 8, T], BF16)
        vext = pool_ve.tile([128, TB, HPC * 65], BF16)
        nc.vector.memset(
            vext[:].rearrange("p tb (h s) -> p tb h s", s=65)[:, :, :, 64], 1.0
        )
        yT = pool_y.tile([128, CS // 128, T], BF16)
        outT_ap = outT_d.ap().rearrange("(co p) t -> p co t", p=128)

        # ---------------- fill task emitters (independent PE work) --------
        def emit_qk(m, tch):
            ps = ps_sm.tile([128, 512], F32, tag="sm", name="qkps")
            for ci in range(NC_):
                nc.tensor.matmul(
                    ps[:], wqk[:, m, ci], xT[:, ci, ts(tch, 512)],
                    start=(ci == 0), stop=(ci == NC_ - 1),
                )
            nc.vector.tensor_scalar_add(
                qkT[:, m, ts(tch, 512)], ps[:], bqk[:, m : m + 1]
            )

        def emit_v(tb):
            ps = ps_sm.tile([128, 512], F32, tag="sm", name="vps")
            for ci in range(NC_):
                nc.tensor.matmul(
                    ps[:], xT[:, ci, ts(tb, 128)], wv[:, ci],
                    start=(ci == 0), stop=(ci == NC_ - 1),
                )
            nc.vector.tensor_copy(
                out=vext[:, tb].rearrange("p (h s) -> p h s", s=65)[:, :, 0:64],
                in_=ps[:].rearrange("p (h d) -> p h d", d=64),
            )

        def emit_proj(co, tch):
            ps = ps_sm.tile([128, 512], F32, tag="sm", name="projps")
            for cit in range(CS // 128):
                nc.tensor.matmul(
                    ps[:], wp[:, cit, ts(co, 128)], yT[:, cit, ts(tch, 512)],
                    start=(cit == 0), stop=(cit == CS // 128 - 1),
                )
            ot = pool_o.tile([128, 512], F32, tag="out")
            nc.vector.tensor_copy(out=ot[:], in_=ps[:])
            nc.sync.dma_start(outT_ap[:, co, ts(tch, 512)], ot[:])

        # fill queues: base = qk m-tiles + V blocks (v gated one chunk ahead
        # so late attention chunks keep fill work); proj gated on chunk
        # completion and preferred once available.
        fq_base = deque()
        for m in (1, 5, 2, 6, 3, 7):
            for tch in range(TCH):
                fq_base.append(("qk", (m, tch), 4096))
        for tb in range(4, TB):
            fq_base.append(("v", (tb,), 4096))
        fq_proj = deque()
        for tch in range(TCH):
            for co in range(C // 128):
                fq_proj.append(("proj", (co, tch), 2048))

        chunks_done = [0] * TCH  # pairs completed per query chunk
        state = {"debt": 0, "qk": {(m, t) for m in (0, 4) for t in range(TCH)},
                 "v": 3, "I": 0}

        def emit_fill(task):
            kind, args, pe = task
            if kind == "qk":
                emit_qk(*args)
                state["qk"].add(args)
            elif kind == "v":
                emit_v(*args)
                state["v"] = max(state["v"], args[0])
            else:
                emit_proj(*args)
            state["debt"] = max(state["debt"] - pe, -12288)

        def pop_fill_while_debt():
            while state["debt"] > 0:
                if fq_proj and chunks_done[fq_proj[0][1][1]] == 4:
                    emit_fill(fq_proj.popleft())
                elif fq_base and (
                    fq_base[0][0] != "v"
                    or fq_base[0][1][0] <= 4 * state["I"] + 7
                ):
                    emit_fill(fq_base.popleft())
                else:
                    break

        def force_prereqs(p, I):
            state["I"] = I
            need = {(m, t) for m in (p, 4 + p) for t in range(TCH)}
            while (not need <= state["qk"]) or state["v"] < 4 * I + 3:
                emit_fill(fq_base.popleft())

        # ---------------- attention chunk (head pair p, query chunk I) ----
        def att_chunk(p, I):
            q0 = I * 512
            psy = [
                ps_y.tile([65, 512], F32, tag="psy", name=f"psy{hb}")
                for hb in range(2)
            ]

            def scores_off(g):
                # 2 full off-diagonal key blocks j = 2g, 2g+1
                pss, Ptl = [], []
                for hb in range(2):
                    s = ps_big.tile([128, 1024], F32, tag="big", name=f"pss{hb}")
                    for jj in range(2):
                        j = 2 * g + jj
                        nc.tensor.matmul(
                            s[:, ts(jj, 512)],
                            qkT[hb * 64 : hb * 64 + 64, 4 + p, ts(j, 128)],
                            qkT[hb * 64 : hb * 64 + 64, p, ds(q0, 512)],
                        )
                    pss.append(s)
                for hb in range(2):
                    P = pool_p.tile([128, 1024], BF16, tag="P", name=f"P{hb}")
                    nc.scalar.activation(P[:], pss[hb][:], EXP, scale=float(SCALE))
                    Ptl.append(P)
                return Ptl

            def av_off(g, Ptl):
                for hb in range(2):
                    h = 2 * p + hb
                    for jj in range(2):
                        j = 2 * g + jj
                        nc.tensor.matmul(
                            psy[hb][:],
                            vext[:, j, ds(h * 65, 65)],
                            Ptl[hb][:, ts(jj, 512)],
                            start=(I > 0 and j == 0),
                            stop=False,
                        )

            def scores_diag_a():
                # diagonal block r=0: full 512 queries
                pss, Ptl = [], []
                for hb in range(2):
                    s = ps_sm.tile([128, 512], F32, tag="sm", name=f"dsa{hb}")
                    nc.tensor.matmul(
                        s[:],
                        qkT[hb * 64 : hb * 64 + 64, 4 + p, ts(4 * I, 128)],
                        qkT[hb * 64 : hb * 64 + 64, p, ds(q0, 512)],
                        start=True, stop=True,
                    )
                    pss.append(s)
                for hb in range(2):
                    P = pool_p.tile([128, 1024], BF16, tag="P", name=f"Pa{hb}")
                    nc.scalar.activation(
                        P[:, 0:512], pss[hb][:], EXP, scale=float(SCALE)
                    )
                    nc.vector.tensor_tensor(
                        P[:, 0:128], P[:, 0:128], tri[:], MULT
                    )
                    Ptl.append(P)
                return Ptl

            def av_diag_a(Ptl):
                for hb in range(2):
                    h = 2 * p + hb
                    nc.tensor.matmul(
                        psy[hb][:],
                        vext[:, 4 * I, ds(h * 65, 65)],
                        Ptl[hb][:, 0:512],
                        start=(I == 0), stop=False,
                    )

            def scores_diag_b():
                # diagonal blocks r=1..3, widths 384/256/128.  Offsets are
                # bank-aligned (0 / 512 / 768): one matmul output must not
                # straddle a 512-fp32 PSUM bank boundary.
                offs = (0, 512, 768)
                wids = (384, 256, 128)
                pss, Ptl = [], []
                for hb in range(2):
                    s = ps_big.tile([128, 1024], F32, tag="big", name=f"dsb{hb}")
                    for r in (1, 2, 3):
                        nc.tensor.matmul(
                            s[:, ds(offs[r - 1], wids[r - 1])],
                            qkT[hb * 64 : hb * 64 + 64, 4 + p, ts(4 * I + r, 128)],
                            qkT[hb * 64 : hb * 64 + 64, p,
                                ds(q0 + 128 * r, wids[r - 1])],
                            start=True, stop=True,
                        )
                    pss.append(s)
                for hb in range(2):
                    P = pool_p.tile([128, 1024], BF16, tag="P", name=f"Pb{hb}")
                    nc.scalar.activation(
                        P[:, 0:384], pss[hb][:, 0:384], EXP, scale=float(SCALE)
                    )
                    nc.scalar.activation(
                        P[:, 512:896], pss[hb][:, 512:896], EXP,
                        scale=float(SCALE),
                    )
                    for o in offs:
                        nc.vector.tensor_tensor(
                            P[:, ds(o, 128)], P[:, ds(o, 128)], tri[:], MULT
                        )
                    Ptl.append(P)
                return Ptl

            def av_diag_b(Ptl):
                offs = (0, 512, 768)
                wids = (384, 256, 128)
                for hb in range(2):
                    h = 2 * p + hb
                    for r in (1, 2, 3):
                        nc.tensor.matmul(
                            psy[hb][:, ds(128 * r, wids[r - 1])],
                            vext[:, 4 * I + r, ds(h * 65, 65)],
                            Ptl[hb][:, ds(offs[r - 1], wids[r - 1])],
                            start=False, stop=(r == 3),
                        )

            # software pipeline: scores(g+1) emitted before av(g); fill
            # matmuls dripped in whenever ACT exp work outruns PE work.
            seq = []
            for g in range(2 * I):
                seq.append((lambda g=g: scores_off(g),
                            lambda Ptl, g=g: av_off(g, Ptl), 4096, 5504))
            seq.append((scores_diag_a, av_diag_a, 2048, 3456))
            seq.append((scores_diag_b, av_diag_b, 3072, 4480))

            pend = None  # (av_fn, Ptl)
            for s_fn, a_fn, pe_c, act_c in seq:
                Ptl = s_fn()
                state["debt"] += act_c - pe_c
                pop_fill_while_debt()
                if pend is not None:
                    pend[0](pend[1])
                pend = (a_fn, Ptl)
            pend[0](pend[1])

            # epilogue: normalize by the ones-column denominator, write yT
            for hb in range(2):
                psb = pool_sb.tile([65, 512], F32R, tag="psb", name=f"psb{hb}")
                nc.vector.tensor_copy(out=psb[:], in_=psy[hb][:])
                rec = pool_rc.tile([1, 512], F32R, tag="rec")
                nc.vector.reciprocal(rec[:], psb[64:65, :])
                bc = pool_bc.tile([64, 512], F32R, tag="bc")
                nc.gpsimd.partition_broadcast(bc[:], rec[:])
                if hb == 0:
                    nc.vector.tensor_tensor(
                        yT[0:64, p, ds(q0, 512)], psb[0:64, :], bc[:], MULT
                    )
                else:
                    yn = pool_yn.tile([64, 512], BF16, tag="yn")
                    nc.vector.tensor_tensor(yn[:], psb[0:64, :], bc[:], MULT)
                    nc.sync.dma_start(yT[64:128, p, ds(q0, 512)], yn[:])

        # ---------------- emission schedule ----------------
        for m in (0, 4):
            for tch in range(TCH):
                emit_qk(m, tch)
        for tb in range(4):
            emit_v(tb)

        for I in range(TCH):
            for p in range(4):
                force_prereqs(p, I)
                att_chunk(p, I)
                chunks_done[I] += 1
        while fq_base:
            emit_fill(fq_base.popleft())
        while fq_proj:
            emit_fill(fq_proj.popleft())

    nc.compile()
    return nc


def kernel(x, w_qkv, b_qkv, w_proj, b_proj):
    x = np.asarray(x, dtype=np.float32)
    w_qkv = np.asarray(w_qkv, dtype=np.float32)
    b_qkv = np.asarray(b_qkv, dtype=np.float32)
    w_proj = np.asarray(w_proj, dtype=np.float32)
    b_proj = np.asarray(b_proj, dtype=np.float32)

    if "nc" not in _CACHE:
        _CACHE["nc"] = _build_program()
    nc = _CACHE["nc"]

    bf = ml_dtypes.bfloat16
    p_ = np.arange(128)[:, None]
    i_ = np.arange(128)[None, :]
    tri = (i_ >= p_).astype(bf)

    in_maps = []
    for c in range(8):
        b, hg = c // 2, c % 2
        sl = slice(hg * CS, (hg + 1) * CS)
        wq = w_qkv[:, sl]
        wk = w_qkv[:, C + hg * CS : C + (hg + 1) * CS]
        wqk_cat = np.concatenate([wq, wk], axis=1)          # [1024, 1024]
        bqk_cat = np.concatenate(
            [b_qkv[sl], b_qkv[C + hg * CS : C + (hg + 1) * CS]]
        )
        wv = w_qkv[:, 2 * C + hg * CS : 2 * C + (hg + 1) * CS]
        in_maps.append({
            "xT": np.ascontiguousarray(
                x[b].T.reshape(NC_, 128, T).transpose(1, 0, 2)
            ).astype(bf),
            "wqk": np.ascontiguousarray(
                wqk_cat.reshape(NC_, 128, 8, 128).transpose(1, 2, 0, 3)
            ).astype(bf),
            "bqk": np.ascontiguousarray(
                bqk_cat.reshape(8, 128).T
            ).astype(np.float32),
            "wv": np.ascontiguousarray(
                wv.reshape(NC_, 128, CS).transpose(1, 0, 2)
            ).astype(bf),
            "wp": np.ascontiguousarray(
                w_proj[hg * CS : (hg + 1) * CS]
                .reshape(CS // 128, 128, C).transpose(1, 0, 2)
            ).astype(bf),
            "tri": tri,
        })

    _CACHE["in_maps"] = in_maps
    res = run_bass_kernel_spmd(nc, in_maps, core_ids=list(range(8)))

    bias = b_proj + b_qkv[2 * C :] @ w_proj
    out = np.empty((B, T, C), dtype=np.float32)
    for b in range(B):
        out[b] = res.results[2 * b]["outT"].T
        out[b] += res.results[2 * b + 1]["outT"].T
        out[b] += bias
    return out
